# revision 20
# baseline (speedup 1.0000x reference)
import sys

sys.path.insert(0, "/opt/trn_rl_repo")

import numpy as np

P = 128          # partitions / tile edge
D = 128          # model dim
H = 4            # heads
DH = 32          # head dim
NCORES = 8

# Full-problem geometry (N=100000, E=800000). Each core owns NBLK node
# blocks of 128 nodes; every block's incident-edge list is padded to
# TBLK tiles of 128 edges so the SPMD program is uniform across cores.
#
# Sharding strategy (graph-partition, data-parallel over destination
# nodes): the host pre-gathers x[col[e]] per edge (an index-select of
# the inputs, like the one-hot/index tensors below), and each core
# projects k/v PER EDGE TILE on device. This trades ~8x redundant
# projection FLOPs for the elimination of the kv table round-trip and
# the descriptor-generation-bound indirect gather.
NBLK_FULL = 98                      # 98*128 = 12544 own nodes/core
NPAD_FULL = NCORES * NBLK_FULL * P  # 100352 padded nodes


def _channel_perm():
    # torch reshape (N, DH, H): flat channel c = d*H + h. We relayout to
    # h-major c' = h*DH + d by permuting weight rows: perm[c'] = d*H + h.
    cp = np.arange(D)
    return (cp % DH) * H + (cp // DH)


GQ = 3           # q-broadcast tiles per block routed to gpsimd gather


def _build_program(NPAD, NOWN, NBLK, TBLK):
    import concourse.bass as bass
    import concourse.tile as tile
    from concourse import bacc, mybir
    from concourse.masks import make_identity
    from contextlib import ExitStack

    dt = mybir.dt
    f32, f16, bf16, f8 = dt.float32, dt.float16, dt.bfloat16, dt.float8e4
    QT = NOWN // P        # x tiles for q projection (own nodes) == NBLK

    nc = bacc.Bacc("TRN2", target_bir_lowering=False, debug=False,
                   num_devices=NCORES)

    # x ships host-transposed (channel-major) so the contraction dim is
    # already on partitions: no PE transpose needed anywhere.
    xot_d = nc.dram_tensor("xot", [D, NOWN], f16, kind="ExternalInput").ap()
    xgt_d = nc.dram_tensor("xgt", [NBLK, D, TBLK * P], f16,
                           kind="ExternalInput").ap()
    wkv_d = nc.dram_tensor("wkv", [D, 2 * D], f16, kind="ExternalInput").ap()
    wq_d = nc.dram_tensor("wq", [D, D], f16, kind="ExternalInput").ap()
    wo_d = nc.dram_tensor("wo", [D, D], f16, kind="ExternalInput").ap()
    bq_d = nc.dram_tensor("bq", [1, D], f16, kind="ExternalInput").ap()
    bot_d = nc.dram_tensor("bot", [D, 1], f32, kind="ExternalInput").ap()
    selt_d = nc.dram_tensor("selt", [NBLK, P, TBLK * P], f8,
                            kind="ExternalInput").ap()
    seli_d = nc.dram_tensor("seli", [NBLK, P, TBLK * P], f8,
                            kind="ExternalInput").ap()

    ciq_d = nc.dram_tensor("ciq", [P, GQ * NBLK], mybir.dt.int32,
                           kind="ExternalInput").ap()

    # output, channel-major: host transposes back
    out_d = nc.dram_tensor("out", [D, NOWN], f32, kind="ExternalOutput").ap()
    q_d = nc.dram_tensor("q", [NOWN, D], f16).ap()  # q table for gathers

    AF = mybir.ActivationFunctionType
    OP = mybir.AluOpType

    with tile.TileContext(nc) as tc, ExitStack() as ctx:
        res = ctx.enter_context(tc.tile_pool(name="res", bufs=1))
        wkv_sb = res.tile([D, 2 * D], f16, name="wkv_sb")
        wq_sb = res.tile([D, D], f16, name="wq_sb")
        wo_sb = res.tile([D, D], f16, name="wo_sb")
        bq_sb = res.tile([1, D], f16, name="bq_sb")
        bot_sb = res.tile([D, 1], f32, name="bot_sb")
        ones_sb = res.tile([1, P], f16, name="ones_sb")
        ident = res.tile([P, P], f16, name="ident")
        q_all = res.tile([P, QT, D], f16, name="q_all")      # resident q
        ybt_all = res.tile([P, QT, P], f16, name="ybt_all")  # resident y^T
        ciq_sb = res.tile([P, GQ * NBLK], mybir.dt.int32, name="ciq_sb")

        for sb_t, dr_t in [(wkv_sb, wkv_d), (wq_sb, wq_d), (wo_sb, wo_d),
                           (bq_sb, bq_d), (bot_sb, bot_d), (ciq_sb, ciq_d)]:
            nc.sync.dma_start(sb_t[:], dr_t[:])
        nc.vector.memset(ones_sb[:], 1.0)
        make_identity(nc, ident[:])

        # shared PSUM pools; each distinct tile name takes bufs x 1 bank:
        # pa 2 + qx 2 + yp 2 + ep 1 + oc 1 == the full 8 banks
        pa = ctx.enter_context(tc.tile_pool(name="pa", bufs=2, space="PSUM"))
        qx = ctx.enter_context(tc.tile_pool(name="qx", bufs=2, space="PSUM"))
        yp = ctx.enter_context(tc.tile_pool(name="yp", bufs=2, space="PSUM"))
        ep = ctx.enter_context(tc.tile_pool(name="ep", bufs=1, space="PSUM"))
        oc = ctx.enter_context(tc.tile_pool(name="oc", bufs=1, space="PSUM"))

        CH = 16  # x tiles per DMA chunk
        with tc.tile_pool(name="xa", bufs=3) as xa:
            # ---- q projection (own nodes); q stays SBUF-resident.
            for j0 in range(0, QT, CH):
                c = min(CH, QT - j0)
                xo16 = xa.tile([P, c * P], f16, name="xo16")
                nc.sync.dma_start(xo16[:], xot_d[:, j0 * P:(j0 + c) * P])
                for t in range(c):
                    q_ps = qx.tile([P, D], f32, name="qx_ps")
                    nc.tensor.matmul(q_ps[:], lhsT=ones_sb[:], rhs=bq_sb[:],
                                     start=True, stop=False)
                    nc.tensor.matmul(q_ps[:],
                                     lhsT=xo16[:, t * P:(t + 1) * P],
                                     rhs=wq_sb[:], start=False, stop=True)
                    nc.scalar.copy(q_all[:, j0 + t, :], q_ps[:])
            # q table to DRAM for the gpsimd q-broadcast gathers
            nc.scalar.dma_start(
                q_d[:].rearrange("(b p) c -> p b c", p=P), q_all[:])

        # ---- main loop: per-block edge kv projection + scores + agg ----
        with tc.tile_pool(name="eg", bufs=3) as eg:
            for b in range(NBLK):
                xg_b = eg.tile([P, TBLK * P], f16, name="xg_b")
                nc.sync.dma_start(xg_b[:], xgt_d[b, :, :])
                selt_b = eg.tile([P, TBLK * P], f8, name="selt_b")
                nc.sync.dma_start(selt_b[:], selt_d[b, :, :])
                sel_b = eg.tile([P, TBLK * P], f8, name="sel_b")
                nc.sync.dma_start(sel_b[:], seli_d[b, :, :])

                # per-edge k/v projection (lhsT = gathered-x tile)
                kvg = eg.tile([P, TBLK, 2 * D], bf16, name="kvg")
                for t in range(TBLK):
                    kv_ps = pa.tile([P, 2 * D], f32, name="kv_ps")
                    nc.tensor.matmul(kv_ps[:],
                                     lhsT=xg_b[:, t * P:(t + 1) * P],
                                     rhs=wkv_sb[:], start=True, stop=True)
                    if t % 2 == 0:
                        nc.vector.tensor_copy(kvg[:, t, :], kv_ps[:])
                    else:
                        nc.scalar.copy(kvg[:, t, :], kv_ps[:])

                # q broadcast: GQ tiles via gpsimd gather (PE-bound
                # kernel, gpsimd idle), the rest via one-hot matmuls
                qxe = eg.tile([P, TBLK, D], f16, name="qxe")
                for t in range(GQ):
                    nc.gpsimd.indirect_dma_start(
                        out=qxe[:, t, :], out_offset=None, in_=q_d[:],
                        in_offset=bass.IndirectOffsetOnAxis(
                            ap=ciq_sb[:, b * GQ + t:b * GQ + t + 1], axis=0))
                for t in range(GQ, TBLK):
                    qx_ps = qx.tile([P, D], f32, name="qx_ps")
                    nc.tensor.matmul(qx_ps[:],
                                     lhsT=selt_b[:, t * P:(t + 1) * P],
                                     rhs=q_all[:, b, :], start=True, stop=True)
                    nc.scalar.copy(qxe[:, t, :], qx_ps[:])

                # prod + head-reduce + exp + v-weighting
                prod = eg.tile([P, TBLK, D], bf16, name="prod")
                nc.vector.tensor_tensor(out=prod[:], in0=qxe[:],
                                        in1=kvg[:, :, 0:D], op=OP.mult)
                s_b = eg.tile([P, TBLK, H], f32, name="s_b")
                nc.vector.tensor_reduce(
                    out=s_b[:],
                    in_=prod[:].rearrange("p t (h d) -> p t h d", h=H),
                    axis=mybir.AxisListType.X, op=OP.add)
                wext = eg.tile([P, TBLK, D + H], bf16, name="wext")
                nc.scalar.activation(wext[:, :, D:D + H], s_b[:], AF.Exp)
                nc.vector.tensor_tensor(
                    out=wext[:, :, 0:D].rearrange("p t (h d) -> p t h d", h=H),
                    in0=kvg[:, :, D:2 * D].rearrange(
                        "p t (h d) -> p t h d", h=H),
                    in1=wext[:, :, D:D + H].to_broadcast((P, TBLK, H, DH)),
                    op=OP.mult)

                ypre = yp.tile([P, D + H], f32, name="ypre")
                for t in range(TBLK):
                    nc.tensor.matmul(ypre[:],
                                     lhsT=sel_b[:, t * P:(t + 1) * P],
                                     rhs=wext[:, t, :],
                                     start=(t == 0), stop=(t == TBLK - 1))

                zr = eg.tile([P, H], f32, name="zr")
                nc.vector.tensor_scalar_add(zr[:], ypre[:, D:D + H], 1e-30)
                rz = eg.tile([P, H], f32, name="rz")
                nc.vector.reciprocal(rz[:], zr[:])
                yb = eg.tile([P, D], f16, name="yb")
                nc.vector.tensor_tensor(
                    out=yb[:].rearrange("p (h d) -> p h d", h=H),
                    in0=ypre[:, 0:D].rearrange("p (h d) -> p h d", h=H),
                    in1=rz[:].to_broadcast((P, H, DH)),
                    op=OP.mult)
                yT_ps = ep.tile([P, D], f16, name="yT_ps")
                nc.tensor.transpose(yT_ps[:], yb[:], ident[:])
                nc.scalar.copy(ybt_all[:, b, :], yT_ps[:])

        # ---- phase C: batched output projection (wo stationary) ----
        CB = 4  # blocks per output matmul (512 node columns)
        with tc.tile_pool(name="ob", bufs=3) as ob:
            for g0 in range(0, QT, CB):
                c = min(CB, QT - g0)
                o_ps = oc.tile([P, c * P], f32, name="o_ps")
                nc.tensor.matmul(
                    o_ps[:], lhsT=wo_sb[:],
                    rhs=ybt_all[:, g0:g0 + c, :].rearrange("p c n -> p (c n)"),
                    start=True, stop=True)
                o_sb = ob.tile([P, c * P], f32, name="o_sb")
                nc.scalar.activation(o_sb[:], o_ps[:], AF.Identity,
                                     bias=bot_sb[:, 0:1])
                nc.scalar.dma_start(out_d[:, g0 * P:(g0 + c) * P], o_sb[:])

    nc.compile()
    return nc


def _prepare_inputs(x, row, col, Wq, bq, Wk, bk, Wv, bv, Wo, bo,
                    NPAD, NOWN, NBLK, TBLK):
    """Host-side sharding: per-core gathered-x, one-hots, weights."""
    N = x.shape[0]
    perm = _channel_perm()
    s = np.sqrt(float(H))
    wkv_in = np.ascontiguousarray(
        np.concatenate([Wk[perm, :].T, Wv[perm, :].T], axis=1)
    ).astype(np.float16)
    wq_in = np.ascontiguousarray((Wq[perm, :] / s).T).astype(np.float16)
    wo_in = np.ascontiguousarray(Wo[:, perm].T).astype(np.float16)
    bq_in = (bq[perm] / s).reshape(1, D).astype(np.float16)
    # bv folds through the output projection exactly: sum_e a_e = 1.
    bot_in = (bo + Wo @ bv).reshape(D, 1).astype(np.float32)

    x_pad = np.zeros((NPAD, D), np.float16)
    x_pad[:N] = x.astype(np.float16)

    NT = NBLK * TBLK
    EPC = NT * P  # padded edges per core
    in_maps = []
    for c in range(NCORES):
        lo, hi = c * NOWN, (c + 1) * NOWN
        e0 = np.searchsorted(row, lo, "left")
        e1 = np.searchsorted(row, hi, "left")
        rows_c = (row[e0:e1] - lo).astype(np.int64)
        cols_c = col[e0:e1].astype(np.int64)
        blk = rows_c // P
        blk_starts = np.searchsorted(blk, np.arange(NBLK), "left")
        rank = np.arange(rows_c.shape[0]) - blk_starts[blk]
        cnts = np.bincount(blk, minlength=NBLK)
        if cnts.max() > TBLK * P:
            raise ValueError(f"TBLK={TBLK} too small: need "
                             f"{int(np.ceil(cnts.max() / P))}")
        pos = blk * (TBLK * P) + rank
        colfull = np.zeros(EPC, np.int64)
        colfull[pos] = cols_c
        # gathered x per edge, block-transposed: [NBLK, D, TBLK*P]
        xgt = np.ascontiguousarray(
            x_pad[colfull].reshape(NBLK, TBLK * P, D).transpose(0, 2, 1))
        jrow = rows_c % P
        import ml_dtypes
        selt = np.zeros((NBLK, P, TBLK * P), ml_dtypes.float8_e4m3)
        selt[blk, jrow, rank] = 1.0
        seli = np.zeros((NBLK, P, TBLK * P), ml_dtypes.float8_e4m3)
        lane_t = rank // P
        lane_p = rank % P
        seli[blk, lane_p, lane_t * P + jrow] = 1.0
        rowfull = np.zeros(EPC, np.int64)
        rowfull[pos] = rows_c
        ciq = rowfull.reshape(NBLK, TBLK, P)[:, :GQ, :]
        ciq = np.ascontiguousarray(
            ciq.reshape(NBLK * GQ, P).T).astype(np.int32)
        in_maps.append({
            "xot": np.ascontiguousarray(x_pad[lo:hi].T),
            "xgt": xgt,
            "wkv": wkv_in, "wq": wq_in, "wo": wo_in,
            "bq": bq_in, "bot": bot_in,
            "selt": selt, "seli": seli, "ciq": ciq,
        })
    return in_maps


def _required_tblk(row, NOWN, NBLK):
    row = np.asarray(row, np.int64)
    need = 1
    for c in range(NCORES):
        lo, hi = c * NOWN, (c + 1) * NOWN
        e0 = np.searchsorted(row, lo, "left")
        e1 = np.searchsorted(row, hi, "left")
        blk = (row[e0:e1] - lo) // P
        cnts = np.bincount(blk, minlength=NBLK)
        need = max(need, int(np.ceil(cnts.max() / P)))
    return need


def _install_ntff_hook():
    """The agent image's antenv lacks axon_hooks; inject it so trace=True
    can drive NTFF profiling through libaxon_pjrt.so."""
    import importlib
    try:
        importlib.import_module("antenv.axon_hooks")
        return
    except ImportError:
        pass
    import types
    if "/root/.axon_site" not in sys.path:
        sys.path.insert(0, "/root/.axon_site")
    from trn_agent_boot.trn_boot import _ntff_profile_via_ctypes
    hook = _ntff_profile_via_ctypes("/opt/axon/libaxon_pjrt.so")
    mod = types.ModuleType("antenv.axon_hooks")
    state = {"hook": hook}
    mod.get_axon_ntff_profile_hook = lambda: state["hook"]
    mod.set_axon_ntff_profile_hook = lambda h: state.update(hook=h)
    import antenv
    antenv.axon_hooks = mod
    sys.modules["antenv.axon_hooks"] = mod


def run(x, row, col, Wq, bq, Wk, bk, Wv, bv, Wo, bo, NBLK=NBLK_FULL,
        trace=False, tmpdir=None):
    from concourse import bass_utils
    from concourse.bass_utils import run_bass_kernel_spmd
    if trace:
        _install_ntff_hook()
        bass_utils.upload_artifacts = lambda d: "local://" + d

    x = np.asarray(x, np.float32)
    row = np.asarray(row, np.int64)
    col = np.asarray(col, np.int64)
    N = x.shape[0]
    NOWN = NBLK * P
    NPAD = NCORES * NOWN
    assert NPAD >= N
    TBLK = _required_tblk(row, NOWN, NBLK)
    nc = _build_program(NPAD, NOWN, NBLK, TBLK)
    in_maps = _prepare_inputs(
        x, row, col,
        np.asarray(Wq, np.float32), np.asarray(bq, np.float32),
        np.asarray(Wk, np.float32), np.asarray(bk, np.float32),
        np.asarray(Wv, np.float32), np.asarray(bv, np.float32),
        np.asarray(Wo, np.float32), np.asarray(bo, np.float32),
        NPAD, NOWN, NBLK, TBLK)
    res = run_bass_kernel_spmd(nc, in_maps, list(range(NCORES)), trace=trace,
                               tmpdir=tmpdir)
    out = np.concatenate(
        [np.ascontiguousarray(res.results[c]["out"].T) for c in range(NCORES)],
        0)
    return out[:N].astype(np.float32), res


def kernel(**inputs):
    out, _ = run(**inputs)
    return out
